# revision 35
# baseline (speedup 1.0000x reference)
"""Trainium2 Bass kernel for nn_Attention_48661979463892.

Multi-head attention: B=2, H=8, dk=dv=64, T=S=2048, E=512.
  keys    = Wk @ x[b]          -> per head [64, T]
  values  = Wv @ x[b]          -> per head [64, T]
  queries = Wq @ y[b]          -> per head [64, S]
  scores  = keys^T @ queries + mask            [T, S]
  attn    = softmax(0.125 * scores, axis=T)    (normalize over keys axis)
  out     = values @ attn                      [64, S]
  res     = W @ concat_heads(out) + b          -> [B, S, O]

Sharding: 16 (batch, head) pairs over 8 cores -> core c handles batch c//4,
head-pair c%4 (global head rows 128*(c%4) .. +128).  Each core emits a
bf16 partial [S, O] contribution of the final linear (its 128 v-channels);
the host sums 4 partials per batch in f32 and adds the bias.

On-device layout per core:
  scores are computed tile-wise as [t_tile=128, s_chunk=512] blocks (both
  heads sharing one [128, 1024] PSUM pair) so softmax's reduce axis (t) is
  the PSUM accumulation axis of the AV matmul; the softmax denominator
  comes from a ones-column appended to values^T (M=65 AV matmul).  The
  1/colsum division is deferred past the per-head final linear: tiny K=1
  matmuls transpose each [1, 128] colsum slice into a PSUM column, one
  [128, 8] reciprocal inverts them, and the per-partition scales are fused
  into the PSUM->SBUF drain of the final-linear results (tensor_scalar +
  scalar_tensor_tensor).  The normalize of chunk sc runs at tile 1 of
  chunk sc+1 and the four per-st final-linear/store pieces at tiles 3-6,
  so only the last chunk's epilogue is exposed in the tail.
Startup: inputs arrive in 256-512KB blocks over three DMA queues (~150
GB/s each, FIFO); the projection pairs are emitted up front and the tile
scheduler dispatches each matmul as its block lands.  Dummy warm-up
matmuls keep the PE p-state ramped while the first blocks arrive.
"""

import numpy as np

N_CORES = 8
B, I, T, S, O = 2, 512, 2048, 2048, 512
H_PER_CORE = 2
DK = 64
SCALING = DK ** -0.5  # 0.125

MM_DTYPE = "bf16"
N_WARMUP_MM = 14

_BUILD_CACHE = {}


def _split_multi_waits(nc):
    """walrus in this toolchain accepts only ONE sync wait per instruction.
    Hoist extra waits onto same-engine NoOps inserted just before."""
    import concourse.mybir as mybir

    ctr = 0
    for fn in nc.m.functions:
        for blk in fn.blocks:
            new_insts = []
            for inst in blk.instructions:
                si = inst.sync_info
                if si is not None and len(si.on_wait) > 1:
                    waits = list(si.on_wait)
                    for w in waits[:-1]:
                        ctr += 1
                        nop = mybir.InstNoOp(
                            name=f"waitsplit-{ctr}", ins=[], outs=[]
                        )
                        nop.engine = inst.engine
                        nop.sync_info = mybir.SyncInfo(on_wait=[w], on_update=[])
                        new_insts.append(nop)
                    del si.on_wait[:-1]
                new_insts.append(inst)
            blk.instructions[:] = new_insts


def _build(with_mask):
    import concourse.bass as bass
    import concourse.mybir as mybir
    import concourse.tile as tile
    from concourse.bass import ts, ds

    f32 = mybir.dt.float32
    mmdt = {
        "f32": f32,
        "f32r": mybir.dt.float32r,
        "bf16": mybir.dt.bfloat16,
    }[MM_DTYPE]
    nc = bass.Bass()
    x_p = nc.declare_dram_parameter("x4", [4, 128, T], mmdt, isOutput=False)
    y_p = nc.declare_dram_parameter("y4", [4, 128, 4, 512], mmdt, isOutput=False)
    wk_p = nc.declare_dram_parameter("wkT", [128, 4, 128], mmdt, isOutput=False)
    wv_p = nc.declare_dram_parameter("wvT", [128, 4, 128], mmdt, isOutput=False)
    wq_p = nc.declare_dram_parameter("wqT", [128, 4, 128], mmdt, isOutput=False)
    wc_p = nc.declare_dram_parameter("wcT", [2, 64, O], mmdt, isOutput=False)
    if with_mask:
        mask_p = nc.declare_dram_parameter("maskT", [16, 128, S], f32, isOutput=False)
    res_p = nc.declare_dram_parameter("res", [S, O], mmdt, isOutput=True)

    N_SC = S // 512    # s chunks of 512
    N_TT = T // 128    # t tiles of 128

    with tile.TileContext(nc) as tc:
        with (
            nc.allow_low_precision(reason="bf16 matmul operands / partials"),
            tc.tile_pool(name="consts", bufs=1) as consts,
            tc.tile_pool(name="exps", bufs=4) as exps_pool,
            tc.tile_pool(name="epi", bufs=2) as epi_pool,
            tc.tile_pool(name="osb", bufs=4) as osb_pool,
            tc.tile_pool(name="resout", bufs=4) as res_pool,
            tc.tile_pool(name="ps_scores", bufs=2, space="PSUM") as ps_scores_pool,
            tc.tile_pool(name="ps_acc", bufs=2, space="PSUM") as ps_acc_pool,
            tc.tile_pool(name="ps_misc", bufs=2, space="PSUM") as ps_misc_pool,
        ):
            # dummy matmuls on scratch data keep the PE busy while the input
            # DMAs land, so the p-state is ramped when real work starts
            scratch_sb = consts.tile([128, 512], mmdt)
            nc.vector.memset(scratch_sb, 0.0)
            for w in range(N_WARMUP_MM):
                ps_w = ps_scores_pool.tile([128, 1024], f32, tag="ps_s", name="ps_w")
                nc.tensor.matmul(
                    ps_w[:, 0:512], scratch_sb[:, 0:128], scratch_sb,
                    start=True, stop=True,
                )

            # ---------------- load inputs ----------------
            wk_sb = consts.tile([128, 4, 128], mmdt)
            wv_sb = consts.tile([128, 4, 128], mmdt)
            wq_sb = consts.tile([128, 4, 128], mmdt)
            wc_sb0 = consts.tile([64, O], mmdt)
            wc_sb1 = consts.tile([64, O], mmdt)
            # dependency tracking is tile-granular: one tile per DMA
            # transfer so each consumer waits only for its own block
            x_hb = [
                [
                    consts.tile([128, 1024], mmdt, name=f"x_hb{j}_{hf}")
                    for hf in range(2)
                ]
                for j in range(4)
            ]
            y_nb = [
                consts.tile([128, 4, 512], mmdt, name=f"y_nb{n}") for n in range(4)
            ]
            # 3 DMA queues (sync/gpsimd/scalar), ~150GB/s each, FIFO per
            # queue; x split into (j, half-T) 256KB blocks so the keys
            # projection streams in arrival order, y chunks contiguous.
            nc.scalar.dma_start(out=y_nb[0], in_=y_p[0])
            nc.sync.dma_start(out=wk_sb, in_=wk_p[:, :, :])
            nc.gpsimd.dma_start(out=wq_sb, in_=wq_p[:, :, :])
            nc.sync.dma_start(out=x_hb[0][0], in_=x_p[0][:, 0:1024])
            nc.gpsimd.dma_start(out=x_hb[3][0], in_=x_p[3][:, 0:1024])
            nc.sync.dma_start(out=x_hb[1][0], in_=x_p[1][:, 0:1024])
            nc.scalar.dma_start(out=x_hb[2][0], in_=x_p[2][:, 0:1024])
            nc.gpsimd.dma_start(out=x_hb[3][1], in_=x_p[3][:, 1024:2048])
            nc.scalar.dma_start(out=wv_sb, in_=wv_p[:, :, :])
            nc.sync.dma_start(out=x_hb[0][1], in_=x_p[0][:, 1024:2048])
            nc.sync.dma_start(out=x_hb[1][1], in_=x_p[1][:, 1024:2048])
            nc.scalar.dma_start(out=x_hb[2][1], in_=x_p[2][:, 1024:2048])
            nc.gpsimd.dma_start(out=y_nb[3], in_=y_p[3])
            nc.sync.dma_start(out=y_nb[2], in_=y_p[2])
            nc.scalar.dma_start(out=y_nb[1], in_=y_p[1])
            nc.gpsimd.dma_start(out=wc_sb0, in_=wc_p[0])
            nc.gpsimd.dma_start(out=wc_sb1, in_=wc_p[1])

            # ---------------- projections ----------------
            # per-slice tiles again so chunk 0's scores don't wait on the
            # n3 projection drains
            keys_nb = [
                consts.tile([128, 512], mmdt, name=f"keys_nb{n}") for n in range(4)
            ]
            qs_nb = [
                consts.tile([128, 512], mmdt, name=f"qs_nb{n}") for n in range(4)
            ]

            def x_src(j, n):
                return x_hb[j][n // 2][:, ts(n % 2, 512)]

            def x_tt(j, tt):
                return x_hb[j][tt // 8][:, ts(tt % 8, 128)]

            def y_src(j, n):
                return y_nb[n][:, j, :]

            def project2(dst, w_sb, src, n0, fillers=0):
                """project n-slices n0 and n0+1 with the j loop outermost so
                each contraction chunk is consumed as its DMA lands."""
                ps0 = ps_misc_pool.tile([128, 512], f32, tag="misc", name="ps0")
                ps1 = ps_misc_pool.tile([128, 512], f32, tag="misc", name="ps1")
                for j in range(4):
                    for ps, n in ((ps0, n0), (ps1, n0 + 1)):
                        nc.tensor.matmul(
                            ps,
                            w_sb[:, j, :],
                            src(j, n),
                            start=(j == 0),
                            stop=(j == 3),
                        )
                    if j < 3:
                        for w in range(fillers):
                            ps_w = ps_scores_pool.tile(
                                [128, 1024], f32, tag="ps_s", name="ps_w"
                            )
                            nc.tensor.matmul(
                                ps_w[:, 0:512], scratch_sb[:, 0:128], scratch_sb,
                                start=True, stop=True,
                            )
                nc.vector.tensor_copy(out=dst[n0], in_=ps0)
                nc.vector.tensor_copy(out=dst[n0 + 1], in_=ps1)

            # values^T with ones columns: [t_part=128, tt, 130]
            # cols 0:64 head0, col 64 ones, cols 65:129 head1, col 129 ones.
            valT_sb = consts.tile([128, N_TT, 130], mmdt)
            nc.vector.memset(valT_sb[:, :, 64:65], 1.0)
            nc.vector.memset(valT_sb[:, :, 129:130], 1.0)

            def valT_proj(tt):
                ps = ps_misc_pool.tile([128, 128], f32, tag="misc", name="psv")
                for j in range(4):
                    nc.tensor.matmul(
                        ps,
                        x_tt(j, tt),
                        wv_sb[:, j, :],
                        start=(j == 0),
                        stop=(j == 3),
                    )
                nc.vector.tensor_copy(out=valT_sb[:, tt, 0:64], in_=ps[:, 0:64])
                nc.vector.tensor_copy(out=valT_sb[:, tt, 65:129], in_=ps[:, 64:128])

            project2(keys_nb, wk_sb, x_src, 0, fillers=6)
            project2(qs_nb, wq_sb, y_src, 0)
            # keys/queries for the later s/t ranges are interleaved into
            # chunk 0 so the first scores tile isn't queued behind them

            def extra_work(tt):
                if tt == 0:
                    valT_proj(0); valT_proj(1)
                elif tt + 1 < N_TT:
                    valT_proj(tt + 1)
                if tt == 1:
                    project2(keys_nb, wk_sb, x_src, 2)
                elif tt == 5:
                    project2(qs_nb, wq_sb, y_src, 2)

            # ---------------- epilogue ----------------
            def normalize(osb):
                """1/colsum as per-partition columns: transpose each [1,128]
                colsum slice into a PSUM column via a K=1 matmul, then one
                tiny [128, 8] reciprocal.  Column h*4+st holds head h,
                s-subtile st."""
                cs_ps = ps_misc_pool.tile([128, 8], f32, tag="misc", name="cs_ps")
                one_mm = valT_sb[64:65, 0, 64:65]
                for h in range(2):
                    for st in range(4):
                        nc.tensor.matmul(
                            cs_ps[:, h * 4 + st : h * 4 + st + 1],
                            osb[h][64:65, ts(st, 128)],
                            one_mm,
                            start=True,
                            stop=True,
                        )
                rec_col = epi_pool.tile([128, 8], f32, tag="rec", name="rec_col")
                nc.vector.reciprocal(out=rec_col, in_=cs_ps)
                return rec_col

            def epilogue_st(sc, osb, rec_col, st, q_eng, last=False):
                """per-head final linear for s-subtile st, the 1/colsum
                scaling fused into the PSUM->SBUF drain, bf16 store."""
                ps_r0 = ps_misc_pool.tile([128, 512], f32, tag="misc", name="ps_r0")
                ps_r1 = ps_misc_pool.tile([128, 512], f32, tag="misc", name="ps_r1")
                nc.tensor.matmul(
                    ps_r0, osb[0][0:64, ts(st, 128)], wc_sb0,
                    start=True, stop=True,
                )
                nc.tensor.matmul(
                    ps_r1, osb[1][0:64, ts(st, 128)], wc_sb1,
                    start=True, stop=True,
                )
                a_sb = res_pool.tile([128, O], f32, tag="a_sb", name="a_sb")
                if last:
                    nc.scalar.activation(
                        out=a_sb,
                        in_=ps_r0,
                        func=mybir.ActivationFunctionType.Copy,
                        scale=rec_col[:, st : st + 1],
                    )
                else:
                    nc.vector.tensor_scalar_mul(
                        a_sb, ps_r0, rec_col[:, st : st + 1]
                    )
                r_sb = res_pool.tile([128, O], mmdt, tag="r_sb", name="r_sb")
                nc.vector.scalar_tensor_tensor(
                    out=r_sb,
                    in0=ps_r1,
                    scalar=rec_col[:, 4 + st : 5 + st],
                    in1=a_sb,
                    op0=mybir.AluOpType.mult,
                    op1=mybir.AluOpType.add,
                )
                q_eng.dma_start(
                    out=res_p[ds(sc * 512 + st * 128, 128), :], in_=r_sb
                )

            # ---------------- attention main loop ----------------
            def t_loop(sc, prev_osb):
                """scores + exp + AV accumulation for s chunk `sc`; the
                previous chunk's normalize runs after tile 1 and its four
                final-linear/store pieces at tiles 3..6, so their PE/DVE
                work hides inside this chunk's stream."""
                rec_prev = None
                ps_o = [
                    ps_acc_pool.tile([65, 512], f32, tag="av", name=f"ps_o{h}")
                    for h in range(2)
                ]
                for tt in range(N_TT):
                    if sc == 0:
                        extra_work(tt)
                    if prev_osb is not None:
                        if tt == 1:
                            rec_prev = normalize(prev_osb)
                        elif 3 <= tt <= 6:
                            st = tt - 3
                            q_eng = nc.sync if st % 2 == 0 else nc.gpsimd
                            epilogue_st(sc - 1, prev_osb, rec_prev, st, q_eng)
                    ps_s = ps_scores_pool.tile([128, 1024], f32, tag="ps_s", name="ps_s")
                    if with_mask:
                        m_sb = exps_pool.tile([128, 512], f32, tag="mask", name="m_sb")
                        nc.sync.dma_start(out=m_sb, in_=mask_p[tt][:, ts(sc, 512)])
                    for h in range(2):
                        nc.tensor.matmul(
                            ps_s[:, ts(h, 512)],
                            keys_nb[tt // 4][64 * h : 64 * h + 64, ts(tt % 4, 128)],
                            qs_nb[sc][64 * h : 64 * h + 64, :],
                            start=True,
                            stop=True,
                        )
                        if with_mask:
                            nc.vector.tensor_tensor(
                                ps_s[:, ts(h, 512)],
                                ps_s[:, ts(h, 512)],
                                m_sb,
                                mybir.AluOpType.add,
                            )
                    ex = exps_pool.tile([128, 1024], mmdt)
                    nc.scalar.activation(
                        out=ex,
                        in_=ps_s,
                        func=mybir.ActivationFunctionType.Exp,
                        scale=float(SCALING),
                    )
                    for h in range(2):
                        nc.tensor.matmul(
                            ps_o[h],
                            valT_sb[:, tt, 65 * h : 65 * h + 65],
                            ex[:, ts(h, 512)],
                            start=(tt == 0),
                            stop=(tt == N_TT - 1),
                        )
                    if sc == N_SC - 1 and tt >= 10:
                        # keep the HAM utilization clamp at full rate into
                        # the tail (it halves the PE clock when activity
                        # drops, making the exposed epilogue 2x slower)
                        ps_w = ps_scores_pool.tile(
                            [128, 1024], f32, tag="ps_s", name="ps_w"
                        )
                        nc.tensor.matmul(
                            ps_w[:, 0:512], scratch_sb[:, 0:128], scratch_sb,
                            start=True, stop=True,
                        )
                osb = []
                last = sc == N_SC - 1
                for h in range(2):
                    o_un = osb_pool.tile([65, 512], mmdt, tag=f"osb{h}", name=f"osb{h}")
                    if last and h == 1:
                        nc.scalar.copy(o_un, ps_o[h])
                    else:
                        nc.vector.tensor_copy(out=o_un, in_=ps_o[h])
                    osb.append(o_un)
                return osb

            prev_osb = None
            for sc in range(N_SC):
                prev_osb = t_loop(sc, prev_osb)
            # tail: only the last chunk's epilogue is exposed; stores go
            # out on all three DMA queues, drains split ACT/DVE.  Filler
            # matmuls gated on the drained AV output keep the PE activity
            # high so the HAM clamp doesn't halve the clock mid-epilogue.
            def tail_filler():
                ps_w = ps_scores_pool.tile(
                    [128, 1024], f32, tag="ps_s", name="ps_w"
                )
                nc.tensor.matmul(
                    ps_w[:, 0:512], prev_osb[0][0:64, 0:128],
                    scratch_sb[0:64, :], start=True, stop=True,
                )
            rec3 = normalize(prev_osb)
            tail_filler()
            tail_q = [nc.sync, nc.gpsimd, nc.scalar, nc.sync]
            for st in range(4):
                epilogue_st(N_SC - 1, prev_osb, rec3, st, tail_q[st], last=True)
                if st < 3:
                    tail_filler()

    _split_multi_waits(nc)
    return nc


def _get_nc(with_mask):
    key = (with_mask, MM_DTYPE)
    if key not in _BUILD_CACHE:
        _BUILD_CACHE[key] = _build(with_mask)
    return _BUILD_CACHE[key]


def _mm_np_dtype():
    if MM_DTYPE == "bf16":
        import ml_dtypes
        return np.dtype(ml_dtypes.bfloat16)
    return np.dtype(np.float32)


def _make_in_maps(x, y, mask, Wk, Wv, Wq, W, with_mask):
    mdt = _mm_np_dtype()
    in_maps = []
    for c in range(N_CORES):
        bb, hp = divmod(c, 4)
        e_sl = slice(128 * hp, 128 * hp + 128)
        im = {
            "x4": np.ascontiguousarray(
                x[bb].reshape(4, 128, T).astype(mdt)
            ),
            "y4": np.ascontiguousarray(
                y[bb].reshape(4, 128, 4, 512).transpose(2, 1, 0, 3).astype(mdt)
            ),
            "wkT": np.ascontiguousarray(
                Wk[e_sl].T.reshape(4, 128, 128).transpose(1, 0, 2).astype(mdt)
            ),
            "wvT": np.ascontiguousarray(
                Wv[e_sl].T.reshape(4, 128, 128).transpose(1, 0, 2).astype(mdt)
            ),
            "wqT": np.ascontiguousarray(
                Wq[e_sl].T.reshape(4, 128, 128).transpose(1, 0, 2).astype(mdt)
            ),
            "wcT": np.ascontiguousarray(
                np.stack(
                    [
                        W[:, 128 * hp : 128 * hp + 64].T,
                        W[:, 128 * hp + 64 : 128 * hp + 128].T,
                    ]
                ).astype(mdt)
            ),
        }
        if with_mask:
            im["maskT"] = np.ascontiguousarray(mask.reshape(16, 128, S))
        in_maps.append(im)
    return in_maps


def kernel(x, y, mask, Wk, Wv, Wq, W, b):
    from concourse.bass_utils import run_bass_kernel_spmd

    x = np.asarray(x, dtype=np.float32)
    y = np.asarray(y, dtype=np.float32)
    mask = np.asarray(mask, dtype=np.float32)
    Wk = np.asarray(Wk, dtype=np.float32)
    Wv = np.asarray(Wv, dtype=np.float32)
    Wq = np.asarray(Wq, dtype=np.float32)
    W = np.asarray(W, dtype=np.float32)
    b = np.asarray(b, dtype=np.float32)

    with_mask = bool(np.any(mask))
    nc = _get_nc(with_mask)
    in_maps = _make_in_maps(x, y, mask, Wk, Wv, Wq, W, with_mask)

    r = run_bass_kernel_spmd(nc, in_maps, core_ids=list(range(N_CORES)))
    parts = [
        np.asarray(r.results[c]["res"]).astype(np.float32) for c in range(N_CORES)
    ]
    out = np.stack(
        [
            parts[0] + parts[1] + parts[2] + parts[3],
            parts[4] + parts[5] + parts[6] + parts[7],
        ],
        axis=0,
    )
    out += b[None, None, :]
    return out.astype(np.float32)
